# revision 41
# baseline (speedup 1.0000x reference)
"""Trainium2 Bass kernel for pre-LN multi-head GQA attention (B=2, S=2048, H=2048,
NH=16, D=128, NKV=4, causal, RoPE).

Sharding: 8 cores = 2 batches x 4 KV groups. Core c handles batch c//4 and KV head
c%4 (its 4 query heads; Wq/Wk/Wv column-sharded by head, Wo row-sharded). Each core
computes a partial output [S, H]; the host sums the 4 per-batch partials.

v2: software-pipelined emission. Per 512-wide s-chunk qb the projection x-pass
(stats + K/V in pass A, Q heads in pass B, x tiles cached in SBUF between passes)
is woven instruction-by-instruction with the previous chunk's attention so the PE
never drains on the exp/select dependency chains. All output projections (Wo) are
deferred to a drain phase where they fill the last chunk's attention stalls.
Activations/weights are bf16 (PE rate is identical to f32r, but DMA and SBUF
halve); PSUM accumulation stays fp32. Diagonal attention tiles are width-restricted
to >=256 columns (causal saving at full fp32r/bf16 PE rate). LayerNorm is applied
by linearity: proj(y) = a * proj(x) + wsum x b, rstd computed as exp(-0.5*ln(var+eps))
so the ACT engine only ever uses one activation table (no table swaps).
"""

import sys

for p in ("/opt/trn_rl_repo",):
    if p not in sys.path:
        sys.path.append(p)

import numpy as np

import concourse.bass as bass
import concourse.tile as tile
from concourse import bacc
from concourse import mybir
from concourse.masks import make_identity

F32 = mybir.dt.float32
BF16 = mybir.dt.bfloat16
ALU = mybir.AluOpType
ACTF = mybir.ActivationFunctionType

B, S, H = 2, 2048, 2048
NH, D, NKV = 16, 128, 4
G = NH // NKV  # query heads per KV head (= heads per core)
EPS = 1e-6
MIN_WIN, MAX_WIN = 1.0, 10000.0
SCALE = 1.0 / float(np.sqrt(np.float32(D)))
CHUNK = 512
NCH = S // CHUNK  # 4
HC = H // 128  # 16

MM_NS = 213.0  # 512-wide full-rate matmul


def build_program(has_bias: bool) -> bass.Bass:
    nc = bacc.Bacc(
        "TRN2",
        target_bir_lowering=False,
        debug=False,
        enable_asserts=False,
        num_devices=8,
    )
    xT = nc.dram_tensor("xT", [H, S], BF16, kind="ExternalInput").ap()
    # host pre-arranged: [128, HC*G*D], [128, HC*D], [128, G*H]
    wq = nc.dram_tensor("wq", [128, HC * G * D], BF16, kind="ExternalInput").ap()
    wkv = nc.dram_tensor("wkv", [128, 2 * HC * D], BF16, kind="ExternalInput").ap()
    wo = nc.dram_tensor("wo", [128, G * H], BF16, kind="ExternalInput").ap()
    cs_d = nc.dram_tensor("cs_t", [128, 2 * S], F32, kind="ExternalInput").ap()
    cst = nc.dram_tensor("consts", [128, 2 * G + 4], F32, kind="ExternalInput").ap()
    outp = nc.dram_tensor("outp", [S, H], F32, kind="ExternalOutput").ap()

    with tile.TileContext(nc) as tc:
        with (
            tc.tile_pool(name="singles", bufs=1) as singles,
            tc.tile_pool(name="xp", bufs=16) as xp,
            tc.tile_pool(name="wrk", bufs=6) as wrk,
            tc.tile_pool(name="sqp", bufs=4) as sqp,
            tc.tile_pool(name="abp", bufs=8) as abp,
            tc.tile_pool(name="tabs", bufs=2) as tabs,
            tc.tile_pool(name="qp", bufs=9) as qp,
            tc.tile_pool(name="evp", bufs=6) as evp,
            tc.tile_pool(name="ep", bufs=4) as ep,
            tc.tile_pool(name="cp", bufs=16) as cp,
            tc.tile_pool(name="rcp", bufs=2) as rcp,
            tc.tile_pool(name="stg", bufs=6) as stg,
            tc.tile_pool(name="psum", bufs=4, space="PSUM") as psum,
        ):
            # ---- on-chip constants (no DMA) ----
            ones_sb = singles.tile([128, 128], BF16)
            nc.gpsimd.memset(ones_sb, 1.0)
            ident = singles.tile([128, 128], BF16)
            make_identity(nc, ident)
            prot_sb = singles.tile([128, 128], BF16)
            nc.gpsimd.memset(prot_sb, 0.0)
            # +1 at (m, m+64): keep where col - row - 64 != 0 else fill 1
            nc.gpsimd.affine_select(
                out=prot_sb, in_=prot_sb, compare_op=ALU.not_equal, fill=1.0,
                base=-64, pattern=[[1, 128]], channel_multiplier=-1,
            )
            # -1 at (m+64, m): keep where col - row + 64 != 0 else fill -1
            nc.gpsimd.affine_select(
                out=prot_sb, in_=prot_sb, compare_op=ALU.not_equal, fill=-1.0,
                base=64, pattern=[[1, 128]], channel_multiplier=-1,
            )
            eps_sb = singles.tile([128, 1], F32)
            nc.gpsimd.memset(eps_sb, EPS)

            # ---- resident weights ----
            kv_w = singles.tile([128, 2, HC, D], BF16)  # loaded in xpass(0)
            wk_sb = kv_w[:, 0]
            wv_sb = kv_w[:, 1]
            cst_sb = singles.tile([128, 2 * G + 4], F32)
            nc.gpsimd.dma_start(cst_sb, cst)
            wqs_sb = cst_sb[:, 0:G]
            wks_sb = cst_sb[:, G : G + 1]
            wvs_sb = cst_sb[:, G + 1 : G + 2]
            bq_sb = cst_sb[:, G + 2 : 2 * G + 2]
            bk_sb = cst_sb[:, 2 * G + 2 : 2 * G + 3]
            bv_sb = cst_sb[:, 2 * G + 3 : 2 * G + 4]
            wo_sb = singles.tile([128, G, H], BF16)  # DMA emitted in period 1
            kT_sb = singles.tile([128, S], BF16)  # roped K^T, filled per chunk
            v_sb = singles.tile([128, S // 128, D], BF16)  # V natural, per k-tile
            # wq loads per-head on the sync ring, interleaved with x loads.
            wq_sb = singles.tile([128, G, HC, D], BF16)

            # cross-stream state
            qts_all: dict[int, list] = {}  # chunk -> [4 roped Q tiles]
            ctx_all: dict[int, list] = {}  # chunk -> [4 ctx^T bf16 tiles]
            ab_t: dict[int, tuple] = {}  # chunk -> (a_t, b_t)
            xtiles: dict[int, list] = {}  # chunk -> [4 x SBUF tiles]
            ctabs: dict[int, tuple] = {}  # chunk -> (cos, sin)

            def load_x(qb, split_first=False):
                sl = slice(qb * CHUNK, (qb + 1) * CHUNK)
                tiles = []
                for hx in range(4):
                    xt4 = xp.tile([128, 4, CHUNK], BF16, tag="x", name="xt4")
                    src = xT[hx * 512 : (hx + 1) * 512, sl].rearrange(
                        "(j p) s -> p j s", p=128
                    )
                    if split_first and hx == 0:
                        for j in range(4):
                            nc.sync.dma_start(xt4[:, j, :], src[:, j, :])
                    else:
                        nc.sync.dma_start(xt4, src)
                    tiles.append(xt4)
                xtiles[qb] = tiles

            def load_tabs(qb):
                cs = tabs.tile([128, 2, CHUNK], F32, tag="cs")
                nc.gpsimd.dma_start(
                    cs,
                    cs_d.rearrange("p (t s) -> p t s", t=2)[
                        :, :, qb * CHUNK : (qb + 1) * CHUNK
                    ],
                )
                ctabs[qb] = (cs[:, 0], cs[:, 1])

            def rope(out, raw, cos_c, sin_c):
                """out = RoPE(raw) for one [128, CHUNK] bf16 tile."""
                rps = psum.tile([128, CHUNK], F32, tag="pl", name="rps", bufs=2)
                nc.tensor.matmul(rps, prot_sb, raw, start=True, stop=True)
                tmp = wrk.tile([128, CHUNK], F32, tag="wrk", name="ropetmp")
                nc.vector.tensor_mul(tmp, rps, sin_c)
                nc.gpsimd.tensor_mul(out, raw, cos_c)
                nc.vector.tensor_add(out, out, tmp)

            def fixup(dst, src_ps, a_t, b_t, wsum_col, bias_col):
                # dst = a * src + wsum_col x b  (+ bias_col); evicts PSUM
                nc.vector.tensor_mul(dst, src_ps, a_t)
                nc.vector.scalar_tensor_tensor(
                    out=dst, in0=b_t, scalar=wsum_col, in1=dst,
                    op0=ALU.mult, op1=ALU.add,
                )
                if bias_col is not None:
                    nc.vector.tensor_scalar_add(dst, dst, bias_col)

            def _seq(*fns):
                def u():
                    for f in fns:
                        f()
                return u

            # ---------------- LN stats prepass (period 0 filler) ----------------
            def sq_of(idx, xt):
                sq = sqp.tile([128, CHUNK], BF16, tag="sq", name="sq")
                r = idx % 3
                if r == 0:
                    nc.scalar.square(sq, xt)
                elif r == 1:
                    nc.vector.tensor_mul(sq, xt, xt)
                else:
                    nc.gpsimd.tensor_mul(sq, xt, xt)
                return sq

            def prepass_units():
                units = []

                def u0():
                    load_x(0)
                    load_x(1)

                units.append((1.0, u0))
                for c in range(NCH):
                    ps: dict = {}

                    def u_nextload(c=c):
                        wq_r = wq.rearrange(
                            "p (g hc q) -> p g hc q", hc=HC, g=G
                        )
                        if c == 0:
                            load_tabs(0)
                            nc.sync.dma_start(wq_sb[:, 0], wq_r[:, 0])
                            nc.sync.dma_start(wq_sb[:, 1], wq_r[:, 1])
                        elif c == 1:
                            load_x(2)
                            nc.sync.dma_start(wq_sb[:, 2], wq_r[:, 2])
                        elif c == 2:
                            nc.sync.dma_start(wq_sb[:, 3], wq_r[:, 3])
                            load_x(3)

                    def u_alloc(c=c, ps=ps):
                        ps["sum"] = psum.tile(
                            [128, CHUNK], F32, tag="cd", name="pp_sum", bufs=2
                        )
                        ps["ssq"] = psum.tile(
                            [128, CHUNK], F32, tag="cd", name="pp_ssq", bufs=2
                        )

                    units.append((1.0, _seq(u_nextload, u_alloc)))

                    def mk_s(c, hc, ps=ps):
                        def u():
                            xt = xtiles[c][hc // 4][:, hc % 4, :]
                            s0, s1 = hc == 0, hc == HC - 1
                            sq = sq_of(c * HC + hc, xt)
                            nc.tensor.matmul(
                                ps["sum"], ones_sb, xt, start=s0, stop=s1
                            )
                            nc.tensor.matmul(
                                ps["ssq"], ones_sb, sq, start=s0, stop=s1
                            )
                        return u

                    units += [(2 * MM_NS, mk_s(c, hc)) for hc in range(HC)]

                    def u_chain(c=c, ps=ps):
                        mean = wrk.tile([128, CHUNK], F32, tag="wrk", name="mean")
                        nc.vector.tensor_scalar_mul(mean, ps["sum"], 1.0 / H)
                        msq = wrk.tile([128, CHUNK], F32, tag="wrk", name="msq")
                        nc.vector.tensor_mul(msq, mean, mean)
                        var = wrk.tile([128, CHUNK], F32, tag="wrk", name="var")
                        nc.vector.scalar_tensor_tensor(
                            out=var, in0=ps["ssq"], scalar=1.0 / H, in1=msq,
                            op0=ALU.mult, op1=ALU.subtract,
                        )
                        lnv = wrk.tile([128, CHUNK], F32, tag="wrk", name="lnv")
                        nc.scalar.activation(lnv, var, ACTF.Ln, bias=eps_sb)
                        a_t = abp.tile([128, CHUNK], F32, tag="ab", name="a_t")
                        nc.scalar.activation(a_t, lnv, ACTF.Exp, scale=-0.5)
                        b_t = abp.tile([128, CHUNK], F32, tag="ab", name="b_t")
                        nc.vector.scalar_tensor_tensor(
                            out=b_t, in0=mean, scalar=-1.0, in1=a_t,
                            op0=ALU.mult, op1=ALU.mult,
                        )
                        ab_t[c] = (a_t, b_t)

                    units.append((1.0, u_chain))
                return units

            # ---------------- x-pass (filler stream F) ----------------
            def xpass_units(qb):
                sl = slice(qb * CHUNK, (qb + 1) * CHUNK)
                st: dict = {}

                def u_dma():
                    if qb == 0:
                        nc.gpsimd.dma_start(
                            kv_w,
                            wkv.rearrange("p (t hc q) -> p t hc q", t=2, hc=HC),
                        )
                    st["xt"] = xtiles[qb]
                    st["cos"], st["sin"] = ctabs[qb]
                    st["k"] = psum.tile([128, CHUNK], F32, tag="xacc", name="psum_k")
                    st["v"] = psum.tile([128, CHUNK], F32, tag="xacc", name="psum_v")

                units = [(1.0, u_dma)]

                # pass A: K + V projections
                def mk_a(hc):
                    def u():
                        xt = st["xt"][hc // 4][:, hc % 4, :]
                        s0, s1 = hc == 0, hc == HC - 1
                        nc.tensor.matmul(st["k"], wk_sb[:, hc, :], xt, start=s0, stop=s1)
                        nc.tensor.matmul(st["v"], wv_sb[:, hc, :], xt, start=s0, stop=s1)
                    return u

                units += [(2 * MM_NS, mk_a(hc)) for hc in range(HC)]

                def u_prefetch():
                    if qb + 1 < NCH:
                        load_tabs(qb + 1)
                    if qb == 1:
                        nc.gpsimd.dma_start(
                            wo_sb, wo.rearrange("p (g h) -> p g h", g=G)
                        )

                def u_kvfix():
                    a_t, b_t = ab_t[qb]
                    kraw = evp.tile([128, CHUNK], BF16, tag="ev", name="kraw")
                    fixup(kraw, st["k"], a_t, b_t, wks_sb[:, 0:1],
                          bk_sb[:, 0:1] if has_bias else None)
                    vt = evp.tile([128, CHUNK], BF16, tag="ev", name="vt")
                    fixup(vt, st["v"], a_t, b_t, wvs_sb[:, 0:1],
                          bv_sb[:, 0:1] if has_bias else None)
                    st["kraw"], st["vt"] = kraw, vt

                units.append((1.0, _seq(u_prefetch, u_kvfix)))

                # pass B: Q heads (reuses cached x tiles)
                def u_allocq():
                    st["q"] = [
                        psum.tile([128, CHUNK], F32, tag="xacc", name=f"psum_q{g_}")
                        for g_ in range(G)
                    ]

                units.append((1.0, u_allocq))

                # head-major: head g's accumulation completes at the g-th
                # quarter of pass B, so its fixup+rope overlaps the rest of B
                # and qts are ready well before the next period's attention.
                def mk_b(g_, hc4):
                    def u():
                        for hc in range(hc4 * 4, hc4 * 4 + 4):
                            xt = st["xt"][hc // 4][:, hc % 4, :]
                            nc.tensor.matmul(
                                st["q"][g_], wq_sb[:, g_, hc, :], xt,
                                start=hc == 0, stop=hc == HC - 1,
                            )
                    return u

                def mk_q(g_):
                    def u():
                        a_t, b_t = ab_t[qb]
                        raw = evp.tile([128, CHUNK], BF16, tag="ev", name="qraw")
                        fixup(raw, st["q"][g_], a_t, b_t,
                              wqs_sb[:, g_ : g_ + 1],
                              bq_sb[:, g_ : g_ + 1] if has_bias else None)
                        q = qp.tile([128, CHUNK], BF16, tag="q")
                        rope(q, raw, st["cos"], st["sin"])
                        qts_all.setdefault(qb, []).append(q)
                    return u

                def u_kv_pe():
                    # K rope into resident kT, V transpose into resident v_sb
                    rope(kT_sb[:, sl], st["kraw"], st["cos"], st["sin"])
                    pt = psum.tile([128, 4, 128], BF16, tag="pl", name="pt", bufs=2)
                    for j in range(4):
                        nc.tensor.transpose(
                            pt[:, j, :], st["vt"][:, j * 128 : (j + 1) * 128], ident
                        )
                    nc.scalar.copy(v_sb[:, qb * 4 : qb * 4 + 4, :], pt)

                for g_ in range(G):
                    units += [(4 * MM_NS, mk_b(g_, hc4)) for hc4 in range(4)]
                    units.append((MM_NS if g_ else 3 * MM_NS,
                                  mk_q(g_) if g_ else _seq(mk_q(g_), u_kv_pe)))
                return units

            def _interleave(a_units, b_units):
                # round-robin proportional merge preserving each list's order
                out = []
                i = j = 0
                na, nb = len(a_units), len(b_units)
                while i < na or j < nb:
                    if j >= nb or (i < na and i * nb <= j * na):
                        out.append(a_units[i])
                        i += 1
                    else:
                        out.append(b_units[j])
                        j += 1
                return out

            # ---------------- attention (dependent stream D) ----------------
            def attn_units(a, inject=None):
                kmax = 4 * (a + 1)
                all_units = []
                for h in range(G):
                    units = []
                    hs: dict = {}

                    def mk_start(h=h, hs=hs):
                        def u():
                            hs["den"] = psum.tile(
                                [128, CHUNK], F32, tag="cd", name="den", bufs=2
                            )
                            hs["ctx"] = psum.tile(
                                [128, CHUNK], F32, tag="cd", name="ctxp", bufs=2
                            )
                            hs["pl"] = {}
                            hs["e"] = {}
                        return u

                    def owidth(kb):
                        kbloc = kb - 4 * a
                        if kbloc < 0:
                            return 0, CHUNK
                        off = kbloc * 128
                        return off, CHUNK - off

                    def mk_l(kb, h=h, hs=hs):
                        off, w = owidth(kb)

                        def u():
                            pl = psum.tile([128, CHUNK], F32, tag="pl", name="pl", bufs=2)
                            hs["pl"][kb] = pl
                            nc.tensor.matmul(
                                pl[:, off : off + w],
                                kT_sb[:, kb * 128 : (kb + 1) * 128],
                                qts_all[a][h][:, off : off + w],
                                start=True, stop=True,
                            )
                        return u

                    def mk_edc(kb, h=h, hs=hs):
                        off, w = owidth(kb)
                        kbloc = kb - 4 * a
                        s0, s1 = kb == 0, kb == kmax - 1

                        def u():
                            pl = hs["pl"].pop(kb)
                            e = ep.tile([128, CHUNK], BF16, tag="e", name="e")
                            nc.scalar.activation(
                                e[:, off : off + w], pl[:, off : off + w],
                                ACTF.Exp, scale=SCALE,
                            )
                            if kbloc >= 0:
                                nc.gpsimd.affine_select(
                                    out=e[:, off : off + w],
                                    in_=e[:, off : off + w],
                                    compare_op=ALU.is_ge,
                                    fill=0.0,
                                    base=off - kbloc * 128,
                                    pattern=[[1, w]],
                                    channel_multiplier=-1,
                                )
                            nc.tensor.matmul(
                                hs["den"][:, off : off + w], ones_sb,
                                e[:, off : off + w], start=s0, stop=s1,
                            )
                            nc.tensor.matmul(
                                hs["ctx"][:, off : off + w], v_sb[:, kb, :],
                                e[:, off : off + w], start=s0, stop=s1,
                            )
                        return u

                    units.append((1.0, mk_start()))
                    # pipeline: L0 L1 EDC0 L2 EDC1 ... L(kmax-1) EDC(kmax-2) EDC(kmax-1)
                    lws = [owidth(kb)[1] for kb in range(kmax)]
                    units.append((MM_NS * lws[0] / 512, mk_l(0)))
                    if kmax > 1:
                        units.append((MM_NS * lws[1] / 512, mk_l(1)))
                    for kb in range(kmax):
                        units.append((2 * MM_NS * lws[kb] / 512, mk_edc(kb)))
                        if kb + 2 < kmax:
                            units.append(
                                (MM_NS * lws[kb + 2] / 512, mk_l(kb + 2))
                            )

                    def mk_norm(h=h, hs=hs):
                        def u():
                            rec = rcp.tile([128, CHUNK], F32, tag="rec", name="rec")
                            nc.vector.reciprocal(rec, hs["den"])
                            ctx = cp.tile([128, CHUNK], BF16, tag="ctx", name="ctx")
                            nc.vector.tensor_mul(ctx, hs["ctx"], rec)
                            ctx_all.setdefault(a, []).append(ctx)
                        return u

                    units.append((1.0, mk_norm()))
                    if inject is not None and h == G - 1:
                        units = _interleave(units, inject)
                    all_units += units
                return all_units

            # ---------------- output projection (drain filler) ----------------
            def wo_units(c):
                units = []
                for sm in range(4):
                    for nc2 in range(4):
                        ps: dict = {}

                        def mk_u1(sm=sm, nc2=nc2, ps=ps):
                            col = slice(nc2 * 512, (nc2 + 1) * 512)

                            def u():
                                ps["po"] = psum.tile(
                                    [128, CHUNK], F32, tag="xacc", name="po"
                                )
                                for cc in (0, 1):
                                    nc.tensor.matmul(
                                        ps["po"],
                                        ctx_all[c][cc][:, sm * 128 : (sm + 1) * 128],
                                        wo_sb[:, cc, col],
                                        start=cc == 0, stop=False,
                                    )
                            return u

                        def mk_u2(sm=sm, nc2=nc2, ps=ps):
                            col = slice(nc2 * 512, (nc2 + 1) * 512)
                            row = slice(c * CHUNK + sm * 128,
                                        c * CHUNK + (sm + 1) * 128)

                            def u():
                                stage = stg.tile(
                                    [128, CHUNK], F32, tag="o", name="stage"
                                )
                                po = ps["po"]
                                for cc in (2, 3):
                                    nc.tensor.matmul(
                                        po,
                                        ctx_all[c][cc][:, sm * 128 : (sm + 1) * 128],
                                        wo_sb[:, cc, col],
                                        start=False, stop=cc == G - 1,
                                    )
                                if (sm + nc2) % 2:
                                    nc.scalar.copy(stage, po)
                                else:
                                    nc.vector.tensor_copy(stage, po)
                                if (sm + nc2) % 2:
                                    nc.sync.dma_start(outp[row, col], stage)
                                else:
                                    nc.scalar.dma_start(outp[row, col], stage)
                            return u

                        units.append((2 * MM_NS, mk_u1()))
                        units.append((2 * MM_NS, mk_u2()))
                return units

            def wo3_units():
                # chunk 3 split: heads 0-2 accumulate + store while head 3's
                # attention is still running; head 3's contribution is then
                # scatter-added into DRAM via gpsimd accumulate-DMA.
                c = NCH - 1
                w1, w2 = [], []
                for sm in range(4):
                    for nc2 in range(4):

                        def mk_w1(sm=sm, nc2=nc2):
                            col = slice(nc2 * 512, (nc2 + 1) * 512)
                            row = slice(c * CHUNK + sm * 128,
                                        c * CHUNK + (sm + 1) * 128)

                            def u():
                                stage = stg.tile(
                                    [128, CHUNK], F32, tag="o", name="stage"
                                )
                                po = psum.tile([128, CHUNK], F32, tag="xacc",
                                               name="po")
                                for cc in range(G - 1):
                                    nc.tensor.matmul(
                                        po,
                                        ctx_all[c][cc][:, sm * 128 : (sm + 1) * 128],
                                        wo_sb[:, cc, col],
                                        start=cc == 0, stop=cc == G - 2,
                                    )
                                if (sm + nc2) % 2:
                                    nc.scalar.copy(stage, po)
                                else:
                                    nc.vector.tensor_copy(stage, po)
                                nc.sync.dma_start(outp[row, col], stage)
                            return u

                        def mk_w2(sm=sm, nc2=nc2):
                            col = slice(nc2 * 512, (nc2 + 1) * 512)
                            row = slice(c * CHUNK + sm * 128,
                                        c * CHUNK + (sm + 1) * 128)

                            def u():
                                stage = stg.tile(
                                    [128, CHUNK], F32, tag="o", name="stage"
                                )
                                po = psum.tile([128, CHUNK], F32, tag="xacc",
                                               name="po")
                                nc.tensor.matmul(
                                    po,
                                    ctx_all[c][G - 1][:, sm * 128 : (sm + 1) * 128],
                                    wo_sb[:, G - 1, col],
                                    start=True, stop=True,
                                )
                                if (sm + nc2) % 2:
                                    nc.scalar.copy(stage, po)
                                else:
                                    nc.vector.tensor_copy(stage, po)
                                nc.gpsimd.dma_start(
                                    outp[row, col], stage,
                                    accum_op=ALU.add,
                                )
                            return u

                        w1.append((3 * MM_NS, mk_w1()))
                        w2.append((MM_NS, mk_w2()))
                return w1, w2

            # ---------------- proportional weave ----------------
            def weave(dep, fill, prime=4500.0):
                td = sum(u[0] for u in dep) or 1.0
                tf = sum(u[0] for u in fill) or 1.0
                i = j = 0
                ad = af = 0.0
                while i < len(dep) or j < len(fill):
                    if j < len(fill) and (
                        af < prime
                        or i >= len(dep)
                        or ad / td < (af - prime) / tf
                    ):
                        af += fill[j][0]
                        fill[j][1]()
                        j += 1
                    else:
                        ad += dep[i][0]
                        dep[i][1]()
                        i += 1

            # ---------------- schedule ----------------
            weave(xpass_units(0), prepass_units(), prime=8500.0)
            for qb in range(1, NCH):
                weave(attn_units(qb - 1), xpass_units(qb))
            drain_fill = []
            for c in range(NCH - 1):
                drain_fill += wo_units(c)
            weave(attn_units(NCH - 1), drain_fill, prime=1000.0)
            for u in wo_units(NCH - 1):
                u[1]()
    nc.compile()
    return nc


_PROGRAMS: dict[bool, bass.Bass] = {}


def get_program(has_bias: bool) -> bass.Bass:
    if has_bias not in _PROGRAMS:
        _PROGRAMS[has_bias] = build_program(has_bias)
    return _PROGRAMS[has_bias]


def make_in_maps(x, ln_gamma, ln_beta, Wq, Wk, Wv, Wo):
    import ml_dtypes

    BF = ml_dtypes.bfloat16
    x = np.asarray(x, np.float32)
    g = np.asarray(ln_gamma, np.float32)
    be = np.asarray(ln_beta, np.float32)
    Wq = np.asarray(Wq, np.float32)
    Wk = np.asarray(Wk, np.float32)
    Wv = np.asarray(Wv, np.float32)
    Wo = np.asarray(Wo, np.float32)

    Wqg = (Wq * g[:, None]).astype(BF)
    Wkg = (Wk * g[:, None]).astype(BF)
    Wvg = (Wv * g[:, None]).astype(BF)
    Wo_b = Wo.astype(BF)
    bq_full = be @ Wq
    bk_full = be @ Wk
    bv_full = be @ Wv
    # column sums of the bf16-rounded weights (device computes with those)
    wqsum = Wqg.astype(np.float32).sum(axis=0)
    wksum = Wkg.astype(np.float32).sum(axis=0)
    wvsum = Wvg.astype(np.float32).sum(axis=0)
    has_bias = bool(np.any(be != 0.0))

    half = D // 2
    ts = MIN_WIN * (MAX_WIN / MIN_WIN) ** (
        2.0 * np.arange(half, dtype=np.float32) / D
    )
    ang = np.arange(S, dtype=np.float32)[None, :] / ts[:, None].astype(np.float32)
    cos_t = np.cos(ang).astype(np.float32)
    sin_t = np.sin(ang).astype(np.float32)
    cos_t = np.concatenate([cos_t, cos_t], axis=0)  # [128, S]
    sin_t = np.concatenate([sin_t, sin_t], axis=0)
    cs_full = np.ascontiguousarray(np.concatenate([cos_t, sin_t], axis=1))

    xT = [np.ascontiguousarray(x[b].T).astype(BF) for b in range(B)]

    def arrange_w(w, ncol):
        # [H, ncol] -> [128, HC*ncol] matching sbuf [128, HC, ncol]
        return np.ascontiguousarray(
            w.reshape(HC, 128, ncol).transpose(1, 0, 2).reshape(128, HC * ncol)
        )

    in_maps = []
    for c in range(8):
        b, h = divmod(c, NKV)
        qs = slice(h * G * D, (h + 1) * G * D)
        ks = slice(h * D, (h + 1) * D)
        wo_slice = Wo_b[qs, :]  # [G*D, H]
        wo_arr = np.ascontiguousarray(
            wo_slice.reshape(G, 128, H).transpose(1, 0, 2).reshape(128, G * H)
        )
        in_maps.append(
            {
                "xT": xT[b],
                "wq": np.ascontiguousarray(
                    Wqg[:, qs]
                    .reshape(HC, 128, G, D)
                    .transpose(1, 2, 0, 3)
                    .reshape(128, G * HC * D)
                ),
                "wkv": np.ascontiguousarray(
                    np.concatenate(
                        [arrange_w(Wkg[:, ks], D), arrange_w(Wvg[:, ks], D)],
                        axis=1,
                    )
                ),
                "wo": wo_arr,
                "cs_t": cs_full,
                "consts": np.ascontiguousarray(
                    np.concatenate(
                        [
                            wqsum[qs].reshape(G, 128).T,
                            wksum[ks][:, None],
                            wvsum[ks][:, None],
                            bq_full[qs].reshape(G, 128).T,
                            bk_full[ks][:, None],
                            bv_full[ks][:, None],
                        ],
                        axis=1,
                    ).astype(np.float32)
                ),
            }
        )
    return in_maps, has_bias


def kernel(x, ln_gamma, ln_beta, Wq, Wk, Wv, Wo):
    from concourse.bass_utils import run_bass_kernel_spmd

    in_maps, has_bias = make_in_maps(x, ln_gamma, ln_beta, Wq, Wk, Wv, Wo)
    nc = get_program(has_bias)
    res = run_bass_kernel_spmd(nc, in_maps, core_ids=list(range(8)))
    outs = [m["outp"] for m in res.results]
    out = np.empty((B, S, H), np.float32)
    for b in range(B):
        out[b] = (outs[NKV * b] + outs[NKV * b + 1]) + (
            outs[NKV * b + 2] + outs[NKV * b + 3]
        )
    return out


# revision 42
# speedup vs baseline: 1.0034x; 1.0034x over previous
"""Trainium2 Bass kernel for pre-LN multi-head GQA attention (B=2, S=2048, H=2048,
NH=16, D=128, NKV=4, causal, RoPE).

Sharding: 8 cores = 2 batches x 4 KV groups. Core c handles batch c//4 and KV head
c%4 (its 4 query heads; Wq/Wk/Wv column-sharded by head, Wo row-sharded). Each core
computes a partial output [S, H]; the host sums the 4 per-batch partials.

v2: software-pipelined emission. Per 512-wide s-chunk qb the projection x-pass
(stats + K/V in pass A, Q heads in pass B, x tiles cached in SBUF between passes)
is woven instruction-by-instruction with the previous chunk's attention so the PE
never drains on the exp/select dependency chains. All output projections (Wo) are
deferred to a drain phase where they fill the last chunk's attention stalls.
Activations/weights are bf16 (PE rate is identical to f32r, but DMA and SBUF
halve); PSUM accumulation stays fp32. Diagonal attention tiles are width-restricted
to >=256 columns (causal saving at full fp32r/bf16 PE rate). LayerNorm is applied
by linearity: proj(y) = a * proj(x) + wsum x b, rstd computed as exp(-0.5*ln(var+eps))
so the ACT engine only ever uses one activation table (no table swaps).
"""

import sys

for p in ("/opt/trn_rl_repo",):
    if p not in sys.path:
        sys.path.append(p)

import numpy as np

import concourse.bass as bass
import concourse.tile as tile
from concourse import bacc
from concourse import mybir
from concourse.masks import make_identity

F32 = mybir.dt.float32
BF16 = mybir.dt.bfloat16
ALU = mybir.AluOpType
ACTF = mybir.ActivationFunctionType

B, S, H = 2, 2048, 2048
NH, D, NKV = 16, 128, 4
G = NH // NKV  # query heads per KV head (= heads per core)
EPS = 1e-6
MIN_WIN, MAX_WIN = 1.0, 10000.0
SCALE = 1.0 / float(np.sqrt(np.float32(D)))
CHUNK = 512
NCH = S // CHUNK  # 4
HC = H // 128  # 16

MM_NS = 213.0  # 512-wide full-rate matmul


def build_program(has_bias: bool) -> bass.Bass:
    nc = bacc.Bacc(
        "TRN2",
        target_bir_lowering=False,
        debug=False,
        enable_asserts=False,
        num_devices=8,
    )
    xT = nc.dram_tensor("xT", [H, S], BF16, kind="ExternalInput").ap()
    # host pre-arranged: [128, HC*G*D], [128, HC*D], [128, G*H]
    wq = nc.dram_tensor("wq", [128, HC * G * D], BF16, kind="ExternalInput").ap()
    wkv = nc.dram_tensor("wkv", [128, 2 * HC * D], BF16, kind="ExternalInput").ap()
    wo = nc.dram_tensor("wo", [128, G * H], BF16, kind="ExternalInput").ap()
    cs_d = nc.dram_tensor("cs_t", [128, 2 * S], F32, kind="ExternalInput").ap()
    cst = nc.dram_tensor("consts", [128, 2 * G + 4], F32, kind="ExternalInput").ap()
    outp = nc.dram_tensor("outp", [S, H], F32, kind="ExternalOutput").ap()

    with tile.TileContext(nc) as tc:
        with (
            tc.tile_pool(name="singles", bufs=1) as singles,
            tc.tile_pool(name="xp", bufs=16) as xp,
            tc.tile_pool(name="wrk", bufs=6) as wrk,
            tc.tile_pool(name="sqp", bufs=4) as sqp,
            tc.tile_pool(name="abp", bufs=8) as abp,
            tc.tile_pool(name="tabs", bufs=2) as tabs,
            tc.tile_pool(name="qp", bufs=9) as qp,
            tc.tile_pool(name="evp", bufs=6) as evp,
            tc.tile_pool(name="ep", bufs=4) as ep,
            tc.tile_pool(name="cp", bufs=16) as cp,
            tc.tile_pool(name="rcp", bufs=2) as rcp,
            tc.tile_pool(name="stg", bufs=6) as stg,
            tc.tile_pool(name="psum", bufs=4, space="PSUM") as psum,
        ):
            # ---- on-chip constants (no DMA) ----
            ones_sb = singles.tile([128, 128], BF16)
            nc.gpsimd.memset(ones_sb, 1.0)
            ident = singles.tile([128, 128], BF16)
            make_identity(nc, ident)
            prot_sb = singles.tile([128, 128], BF16)
            nc.gpsimd.memset(prot_sb, 0.0)
            # +1 at (m, m+64): keep where col - row - 64 != 0 else fill 1
            nc.gpsimd.affine_select(
                out=prot_sb, in_=prot_sb, compare_op=ALU.not_equal, fill=1.0,
                base=-64, pattern=[[1, 128]], channel_multiplier=-1,
            )
            # -1 at (m+64, m): keep where col - row + 64 != 0 else fill -1
            nc.gpsimd.affine_select(
                out=prot_sb, in_=prot_sb, compare_op=ALU.not_equal, fill=-1.0,
                base=64, pattern=[[1, 128]], channel_multiplier=-1,
            )
            eps_sb = singles.tile([128, 1], F32)
            nc.gpsimd.memset(eps_sb, EPS)

            # ---- resident weights ----
            kv_w = singles.tile([128, 2, HC, D], BF16)  # loaded in xpass(0)
            wk_sb = kv_w[:, 0]
            wv_sb = kv_w[:, 1]
            cst_sb = singles.tile([128, 2 * G + 4], F32)
            nc.gpsimd.dma_start(cst_sb, cst)
            wqs_sb = cst_sb[:, 0:G]
            wks_sb = cst_sb[:, G : G + 1]
            wvs_sb = cst_sb[:, G + 1 : G + 2]
            bq_sb = cst_sb[:, G + 2 : 2 * G + 2]
            bk_sb = cst_sb[:, 2 * G + 2 : 2 * G + 3]
            bv_sb = cst_sb[:, 2 * G + 3 : 2 * G + 4]
            wo_sb = singles.tile([128, G, H], BF16)  # DMA emitted in period 1
            kT_sb = singles.tile([128, S], BF16)  # roped K^T, filled per chunk
            v_sb = singles.tile([128, S // 128, D], BF16)  # V natural, per k-tile
            # wq loads per-head on the sync ring, interleaved with x loads.
            wq_sb = singles.tile([128, G, HC, D], BF16)

            # cross-stream state
            qts_all: dict[int, list] = {}  # chunk -> [4 roped Q tiles]
            ctx_all: dict[int, list] = {}  # chunk -> [4 ctx^T bf16 tiles]
            ab_t: dict[int, tuple] = {}  # chunk -> (a_t, b_t)
            xtiles: dict[int, list] = {}  # chunk -> [4 x SBUF tiles]
            ctabs: dict[int, tuple] = {}  # chunk -> (cos, sin)

            def load_x(qb, split_first=False):
                sl = slice(qb * CHUNK, (qb + 1) * CHUNK)
                tiles = []
                for hx in range(4):
                    xt4 = xp.tile([128, 4, CHUNK], BF16, tag="x", name="xt4")
                    src = xT[hx * 512 : (hx + 1) * 512, sl].rearrange(
                        "(j p) s -> p j s", p=128
                    )
                    if split_first and hx == 0:
                        for j in range(4):
                            nc.sync.dma_start(xt4[:, j, :], src[:, j, :])
                    else:
                        nc.sync.dma_start(xt4, src)
                    tiles.append(xt4)
                xtiles[qb] = tiles

            def load_tabs(qb):
                cs = tabs.tile([128, 2, CHUNK], F32, tag="cs")
                nc.gpsimd.dma_start(
                    cs,
                    cs_d.rearrange("p (t s) -> p t s", t=2)[
                        :, :, qb * CHUNK : (qb + 1) * CHUNK
                    ],
                )
                ctabs[qb] = (cs[:, 0], cs[:, 1])

            def rope(out, raw, cos_c, sin_c):
                """out = RoPE(raw) for one [128, CHUNK] bf16 tile."""
                rps = psum.tile([128, CHUNK], F32, tag="pl", name="rps", bufs=2)
                nc.tensor.matmul(rps, prot_sb, raw, start=True, stop=True)
                tmp = wrk.tile([128, CHUNK], F32, tag="wrk", name="ropetmp")
                nc.vector.tensor_mul(tmp, rps, sin_c)
                nc.gpsimd.tensor_mul(out, raw, cos_c)
                nc.vector.tensor_add(out, out, tmp)

            def fixup(dst, src_ps, a_t, b_t, wsum_col, bias_col):
                # dst = a * src + wsum_col x b  (+ bias_col); evicts PSUM
                nc.vector.tensor_mul(dst, src_ps, a_t)
                nc.vector.scalar_tensor_tensor(
                    out=dst, in0=b_t, scalar=wsum_col, in1=dst,
                    op0=ALU.mult, op1=ALU.add,
                )
                if bias_col is not None:
                    nc.vector.tensor_scalar_add(dst, dst, bias_col)

            def _seq(*fns):
                def u():
                    for f in fns:
                        f()
                return u

            # ---------------- LN stats prepass (period 0 filler) ----------------
            def sq_of(idx, xt):
                sq = sqp.tile([128, CHUNK], BF16, tag="sq", name="sq")
                r = idx % 3
                if r == 0:
                    nc.scalar.square(sq, xt)
                elif r == 1:
                    nc.vector.tensor_mul(sq, xt, xt)
                else:
                    nc.gpsimd.tensor_mul(sq, xt, xt)
                return sq

            def prepass_units():
                units = []

                def u0():
                    load_x(0, split_first=True)
                    load_x(1)

                units.append((1.0, u0))
                for c in range(NCH):
                    ps: dict = {}

                    def u_nextload(c=c):
                        wq_r = wq.rearrange(
                            "p (g hc q) -> p g hc q", hc=HC, g=G
                        )
                        if c == 0:
                            load_tabs(0)
                            nc.sync.dma_start(wq_sb[:, 0], wq_r[:, 0])
                            nc.sync.dma_start(wq_sb[:, 1], wq_r[:, 1])
                        elif c == 1:
                            load_x(2)
                            nc.sync.dma_start(wq_sb[:, 2], wq_r[:, 2])
                        elif c == 2:
                            nc.sync.dma_start(wq_sb[:, 3], wq_r[:, 3])
                            load_x(3)

                    def u_alloc(c=c, ps=ps):
                        ps["sum"] = psum.tile(
                            [128, CHUNK], F32, tag="cd", name="pp_sum", bufs=2
                        )
                        ps["ssq"] = psum.tile(
                            [128, CHUNK], F32, tag="cd", name="pp_ssq", bufs=2
                        )

                    units.append((1.0, _seq(u_nextload, u_alloc)))

                    def mk_s(c, hc, ps=ps):
                        def u():
                            xt = xtiles[c][hc // 4][:, hc % 4, :]
                            s0, s1 = hc == 0, hc == HC - 1
                            sq = sq_of(c * HC + hc, xt)
                            nc.tensor.matmul(
                                ps["sum"], ones_sb, xt, start=s0, stop=s1
                            )
                            nc.tensor.matmul(
                                ps["ssq"], ones_sb, sq, start=s0, stop=s1
                            )
                        return u

                    units += [(2 * MM_NS, mk_s(c, hc)) for hc in range(HC)]

                    def u_chain(c=c, ps=ps):
                        mean = wrk.tile([128, CHUNK], F32, tag="wrk", name="mean")
                        nc.vector.tensor_scalar_mul(mean, ps["sum"], 1.0 / H)
                        msq = wrk.tile([128, CHUNK], F32, tag="wrk", name="msq")
                        nc.vector.tensor_mul(msq, mean, mean)
                        var = wrk.tile([128, CHUNK], F32, tag="wrk", name="var")
                        nc.vector.scalar_tensor_tensor(
                            out=var, in0=ps["ssq"], scalar=1.0 / H, in1=msq,
                            op0=ALU.mult, op1=ALU.subtract,
                        )
                        lnv = wrk.tile([128, CHUNK], F32, tag="wrk", name="lnv")
                        nc.scalar.activation(lnv, var, ACTF.Ln, bias=eps_sb)
                        a_t = abp.tile([128, CHUNK], F32, tag="ab", name="a_t")
                        nc.scalar.activation(a_t, lnv, ACTF.Exp, scale=-0.5)
                        b_t = abp.tile([128, CHUNK], F32, tag="ab", name="b_t")
                        nc.vector.scalar_tensor_tensor(
                            out=b_t, in0=mean, scalar=-1.0, in1=a_t,
                            op0=ALU.mult, op1=ALU.mult,
                        )
                        ab_t[c] = (a_t, b_t)

                    units.append((1.0, u_chain))
                return units

            # ---------------- x-pass (filler stream F) ----------------
            def xpass_units(qb):
                sl = slice(qb * CHUNK, (qb + 1) * CHUNK)
                st: dict = {}

                def u_dma():
                    if qb == 0:
                        nc.gpsimd.dma_start(
                            kv_w,
                            wkv.rearrange("p (t hc q) -> p t hc q", t=2, hc=HC),
                        )
                    st["xt"] = xtiles[qb]
                    st["cos"], st["sin"] = ctabs[qb]
                    st["k"] = psum.tile([128, CHUNK], F32, tag="xacc", name="psum_k")
                    st["v"] = psum.tile([128, CHUNK], F32, tag="xacc", name="psum_v")

                units = [(1.0, u_dma)]

                # pass A: K + V projections
                def mk_a(hc):
                    def u():
                        xt = st["xt"][hc // 4][:, hc % 4, :]
                        s0, s1 = hc == 0, hc == HC - 1
                        nc.tensor.matmul(st["k"], wk_sb[:, hc, :], xt, start=s0, stop=s1)
                        nc.tensor.matmul(st["v"], wv_sb[:, hc, :], xt, start=s0, stop=s1)
                    return u

                units += [(2 * MM_NS, mk_a(hc)) for hc in range(HC)]

                def u_prefetch():
                    if qb + 1 < NCH:
                        load_tabs(qb + 1)
                    if qb == 1:
                        nc.gpsimd.dma_start(
                            wo_sb, wo.rearrange("p (g h) -> p g h", g=G)
                        )

                def u_kvfix():
                    a_t, b_t = ab_t[qb]
                    kraw = evp.tile([128, CHUNK], BF16, tag="ev", name="kraw")
                    fixup(kraw, st["k"], a_t, b_t, wks_sb[:, 0:1],
                          bk_sb[:, 0:1] if has_bias else None)
                    vt = evp.tile([128, CHUNK], BF16, tag="ev", name="vt")
                    fixup(vt, st["v"], a_t, b_t, wvs_sb[:, 0:1],
                          bv_sb[:, 0:1] if has_bias else None)
                    st["kraw"], st["vt"] = kraw, vt

                units.append((1.0, _seq(u_prefetch, u_kvfix)))

                # pass B: Q heads (reuses cached x tiles)
                def u_allocq():
                    st["q"] = [
                        psum.tile([128, CHUNK], F32, tag="xacc", name=f"psum_q{g_}")
                        for g_ in range(G)
                    ]

                units.append((1.0, u_allocq))

                # head-major: head g's accumulation completes at the g-th
                # quarter of pass B, so its fixup+rope overlaps the rest of B
                # and qts are ready well before the next period's attention.
                def mk_b(g_, hc4):
                    def u():
                        for hc in range(hc4 * 4, hc4 * 4 + 4):
                            xt = st["xt"][hc // 4][:, hc % 4, :]
                            nc.tensor.matmul(
                                st["q"][g_], wq_sb[:, g_, hc, :], xt,
                                start=hc == 0, stop=hc == HC - 1,
                            )
                    return u

                def mk_q(g_):
                    def u():
                        a_t, b_t = ab_t[qb]
                        raw = evp.tile([128, CHUNK], BF16, tag="ev", name="qraw")
                        fixup(raw, st["q"][g_], a_t, b_t,
                              wqs_sb[:, g_ : g_ + 1],
                              bq_sb[:, g_ : g_ + 1] if has_bias else None)
                        q = qp.tile([128, CHUNK], BF16, tag="q")
                        rope(q, raw, st["cos"], st["sin"])
                        qts_all.setdefault(qb, []).append(q)
                    return u

                def u_kv_pe():
                    # K rope into resident kT, V transpose into resident v_sb
                    rope(kT_sb[:, sl], st["kraw"], st["cos"], st["sin"])
                    pt = psum.tile([128, 4, 128], BF16, tag="pl", name="pt", bufs=2)
                    for j in range(4):
                        nc.tensor.transpose(
                            pt[:, j, :], st["vt"][:, j * 128 : (j + 1) * 128], ident
                        )
                    nc.scalar.copy(v_sb[:, qb * 4 : qb * 4 + 4, :], pt)

                for g_ in range(G):
                    units += [(4 * MM_NS, mk_b(g_, hc4)) for hc4 in range(4)]
                    units.append((MM_NS if g_ else 3 * MM_NS,
                                  mk_q(g_) if g_ else _seq(mk_q(g_), u_kv_pe)))
                return units

            def _interleave(a_units, b_units):
                # round-robin proportional merge preserving each list's order
                out = []
                i = j = 0
                na, nb = len(a_units), len(b_units)
                while i < na or j < nb:
                    if j >= nb or (i < na and i * nb <= j * na):
                        out.append(a_units[i])
                        i += 1
                    else:
                        out.append(b_units[j])
                        j += 1
                return out

            # ---------------- attention (dependent stream D) ----------------
            def attn_units(a, inject=None):
                kmax = 4 * (a + 1)
                all_units = []
                for h in range(G):
                    units = []
                    hs: dict = {}

                    def mk_start(h=h, hs=hs):
                        def u():
                            hs["den"] = psum.tile(
                                [128, CHUNK], F32, tag="cd", name="den", bufs=2
                            )
                            hs["ctx"] = psum.tile(
                                [128, CHUNK], F32, tag="cd", name="ctxp", bufs=2
                            )
                            hs["pl"] = {}
                            hs["e"] = {}
                        return u

                    def owidth(kb):
                        kbloc = kb - 4 * a
                        if kbloc < 0:
                            return 0, CHUNK
                        off = kbloc * 128
                        return off, CHUNK - off

                    def mk_l(kb, h=h, hs=hs):
                        off, w = owidth(kb)

                        def u():
                            pl = psum.tile([128, CHUNK], F32, tag="pl", name="pl", bufs=2)
                            hs["pl"][kb] = pl
                            nc.tensor.matmul(
                                pl[:, off : off + w],
                                kT_sb[:, kb * 128 : (kb + 1) * 128],
                                qts_all[a][h][:, off : off + w],
                                start=True, stop=True,
                            )
                        return u

                    def mk_edc(kb, h=h, hs=hs):
                        off, w = owidth(kb)
                        kbloc = kb - 4 * a
                        s0, s1 = kb == 0, kb == kmax - 1

                        def u():
                            pl = hs["pl"].pop(kb)
                            e = ep.tile([128, CHUNK], BF16, tag="e", name="e")
                            nc.scalar.activation(
                                e[:, off : off + w], pl[:, off : off + w],
                                ACTF.Exp, scale=SCALE,
                            )
                            if kbloc >= 0:
                                nc.gpsimd.affine_select(
                                    out=e[:, off : off + w],
                                    in_=e[:, off : off + w],
                                    compare_op=ALU.is_ge,
                                    fill=0.0,
                                    base=off - kbloc * 128,
                                    pattern=[[1, w]],
                                    channel_multiplier=-1,
                                )
                            nc.tensor.matmul(
                                hs["den"][:, off : off + w], ones_sb,
                                e[:, off : off + w], start=s0, stop=s1,
                            )
                            nc.tensor.matmul(
                                hs["ctx"][:, off : off + w], v_sb[:, kb, :],
                                e[:, off : off + w], start=s0, stop=s1,
                            )
                        return u

                    units.append((1.0, mk_start()))
                    # pipeline: L0 L1 EDC0 L2 EDC1 ... L(kmax-1) EDC(kmax-2) EDC(kmax-1)
                    lws = [owidth(kb)[1] for kb in range(kmax)]
                    units.append((MM_NS * lws[0] / 512, mk_l(0)))
                    if kmax > 1:
                        units.append((MM_NS * lws[1] / 512, mk_l(1)))
                    for kb in range(kmax):
                        units.append((2 * MM_NS * lws[kb] / 512, mk_edc(kb)))
                        if kb + 2 < kmax:
                            units.append(
                                (MM_NS * lws[kb + 2] / 512, mk_l(kb + 2))
                            )

                    def mk_norm(h=h, hs=hs):
                        def u():
                            rec = rcp.tile([128, CHUNK], F32, tag="rec", name="rec")
                            nc.vector.reciprocal(rec, hs["den"])
                            ctx = cp.tile([128, CHUNK], BF16, tag="ctx", name="ctx")
                            nc.vector.tensor_mul(ctx, hs["ctx"], rec)
                            ctx_all.setdefault(a, []).append(ctx)
                        return u

                    units.append((1.0, mk_norm()))
                    if inject is not None and h == G - 1:
                        units = _interleave(units, inject)
                    all_units += units
                return all_units

            # ---------------- output projection (drain filler) ----------------
            def wo_units(c):
                units = []
                for sm in range(4):
                    for nc2 in range(4):
                        ps: dict = {}

                        def mk_u1(sm=sm, nc2=nc2, ps=ps):
                            col = slice(nc2 * 512, (nc2 + 1) * 512)

                            def u():
                                ps["po"] = psum.tile(
                                    [128, CHUNK], F32, tag="xacc", name="po"
                                )
                                for cc in (0, 1):
                                    nc.tensor.matmul(
                                        ps["po"],
                                        ctx_all[c][cc][:, sm * 128 : (sm + 1) * 128],
                                        wo_sb[:, cc, col],
                                        start=cc == 0, stop=False,
                                    )
                            return u

                        def mk_u2(sm=sm, nc2=nc2, ps=ps):
                            col = slice(nc2 * 512, (nc2 + 1) * 512)
                            row = slice(c * CHUNK + sm * 128,
                                        c * CHUNK + (sm + 1) * 128)

                            def u():
                                stage = stg.tile(
                                    [128, CHUNK], F32, tag="o", name="stage"
                                )
                                po = ps["po"]
                                for cc in (2, 3):
                                    nc.tensor.matmul(
                                        po,
                                        ctx_all[c][cc][:, sm * 128 : (sm + 1) * 128],
                                        wo_sb[:, cc, col],
                                        start=False, stop=cc == G - 1,
                                    )
                                if (sm + nc2) % 2:
                                    nc.scalar.copy(stage, po)
                                else:
                                    nc.vector.tensor_copy(stage, po)
                                if (sm + nc2) % 2:
                                    nc.sync.dma_start(outp[row, col], stage)
                                else:
                                    nc.scalar.dma_start(outp[row, col], stage)
                            return u

                        units.append((2 * MM_NS, mk_u1()))
                        units.append((2 * MM_NS, mk_u2()))
                return units

            def wo3_units():
                # chunk 3 split: heads 0-2 accumulate + store while head 3's
                # attention is still running; head 3's contribution is then
                # scatter-added into DRAM via gpsimd accumulate-DMA.
                c = NCH - 1
                w1, w2 = [], []
                for sm in range(4):
                    for nc2 in range(4):

                        def mk_w1(sm=sm, nc2=nc2):
                            col = slice(nc2 * 512, (nc2 + 1) * 512)
                            row = slice(c * CHUNK + sm * 128,
                                        c * CHUNK + (sm + 1) * 128)

                            def u():
                                stage = stg.tile(
                                    [128, CHUNK], F32, tag="o", name="stage"
                                )
                                po = psum.tile([128, CHUNK], F32, tag="xacc",
                                               name="po")
                                for cc in range(G - 1):
                                    nc.tensor.matmul(
                                        po,
                                        ctx_all[c][cc][:, sm * 128 : (sm + 1) * 128],
                                        wo_sb[:, cc, col],
                                        start=cc == 0, stop=cc == G - 2,
                                    )
                                if (sm + nc2) % 2:
                                    nc.scalar.copy(stage, po)
                                else:
                                    nc.vector.tensor_copy(stage, po)
                                nc.sync.dma_start(outp[row, col], stage)
                            return u

                        def mk_w2(sm=sm, nc2=nc2):
                            col = slice(nc2 * 512, (nc2 + 1) * 512)
                            row = slice(c * CHUNK + sm * 128,
                                        c * CHUNK + (sm + 1) * 128)

                            def u():
                                stage = stg.tile(
                                    [128, CHUNK], F32, tag="o", name="stage"
                                )
                                po = psum.tile([128, CHUNK], F32, tag="xacc",
                                               name="po")
                                nc.tensor.matmul(
                                    po,
                                    ctx_all[c][G - 1][:, sm * 128 : (sm + 1) * 128],
                                    wo_sb[:, G - 1, col],
                                    start=True, stop=True,
                                )
                                if (sm + nc2) % 2:
                                    nc.scalar.copy(stage, po)
                                else:
                                    nc.vector.tensor_copy(stage, po)
                                nc.gpsimd.dma_start(
                                    outp[row, col], stage,
                                    accum_op=ALU.add,
                                )
                            return u

                        w1.append((3 * MM_NS, mk_w1()))
                        w2.append((MM_NS, mk_w2()))
                return w1, w2

            # ---------------- proportional weave ----------------
            def weave(dep, fill, prime=4500.0):
                td = sum(u[0] for u in dep) or 1.0
                tf = sum(u[0] for u in fill) or 1.0
                i = j = 0
                ad = af = 0.0
                while i < len(dep) or j < len(fill):
                    if j < len(fill) and (
                        af < prime
                        or i >= len(dep)
                        or ad / td < (af - prime) / tf
                    ):
                        af += fill[j][0]
                        fill[j][1]()
                        j += 1
                    else:
                        ad += dep[i][0]
                        dep[i][1]()
                        i += 1

            # ---------------- schedule ----------------
            weave(xpass_units(0), prepass_units(), prime=8500.0)
            for qb in range(1, NCH):
                weave(attn_units(qb - 1), xpass_units(qb))
            drain_fill = []
            for c in range(NCH - 1):
                drain_fill += wo_units(c)
            weave(attn_units(NCH - 1), drain_fill, prime=1000.0)
            for u in wo_units(NCH - 1):
                u[1]()
    nc.compile()
    return nc


_PROGRAMS: dict[bool, bass.Bass] = {}


def get_program(has_bias: bool) -> bass.Bass:
    if has_bias not in _PROGRAMS:
        _PROGRAMS[has_bias] = build_program(has_bias)
    return _PROGRAMS[has_bias]


def make_in_maps(x, ln_gamma, ln_beta, Wq, Wk, Wv, Wo):
    import ml_dtypes

    BF = ml_dtypes.bfloat16
    x = np.asarray(x, np.float32)
    g = np.asarray(ln_gamma, np.float32)
    be = np.asarray(ln_beta, np.float32)
    Wq = np.asarray(Wq, np.float32)
    Wk = np.asarray(Wk, np.float32)
    Wv = np.asarray(Wv, np.float32)
    Wo = np.asarray(Wo, np.float32)

    Wqg = (Wq * g[:, None]).astype(BF)
    Wkg = (Wk * g[:, None]).astype(BF)
    Wvg = (Wv * g[:, None]).astype(BF)
    Wo_b = Wo.astype(BF)
    bq_full = be @ Wq
    bk_full = be @ Wk
    bv_full = be @ Wv
    # column sums of the bf16-rounded weights (device computes with those)
    wqsum = Wqg.astype(np.float32).sum(axis=0)
    wksum = Wkg.astype(np.float32).sum(axis=0)
    wvsum = Wvg.astype(np.float32).sum(axis=0)
    has_bias = bool(np.any(be != 0.0))

    half = D // 2
    ts = MIN_WIN * (MAX_WIN / MIN_WIN) ** (
        2.0 * np.arange(half, dtype=np.float32) / D
    )
    ang = np.arange(S, dtype=np.float32)[None, :] / ts[:, None].astype(np.float32)
    cos_t = np.cos(ang).astype(np.float32)
    sin_t = np.sin(ang).astype(np.float32)
    cos_t = np.concatenate([cos_t, cos_t], axis=0)  # [128, S]
    sin_t = np.concatenate([sin_t, sin_t], axis=0)
    cs_full = np.ascontiguousarray(np.concatenate([cos_t, sin_t], axis=1))

    xT = [np.ascontiguousarray(x[b].T).astype(BF) for b in range(B)]

    def arrange_w(w, ncol):
        # [H, ncol] -> [128, HC*ncol] matching sbuf [128, HC, ncol]
        return np.ascontiguousarray(
            w.reshape(HC, 128, ncol).transpose(1, 0, 2).reshape(128, HC * ncol)
        )

    in_maps = []
    for c in range(8):
        b, h = divmod(c, NKV)
        qs = slice(h * G * D, (h + 1) * G * D)
        ks = slice(h * D, (h + 1) * D)
        wo_slice = Wo_b[qs, :]  # [G*D, H]
        wo_arr = np.ascontiguousarray(
            wo_slice.reshape(G, 128, H).transpose(1, 0, 2).reshape(128, G * H)
        )
        in_maps.append(
            {
                "xT": xT[b],
                "wq": np.ascontiguousarray(
                    Wqg[:, qs]
                    .reshape(HC, 128, G, D)
                    .transpose(1, 2, 0, 3)
                    .reshape(128, G * HC * D)
                ),
                "wkv": np.ascontiguousarray(
                    np.concatenate(
                        [arrange_w(Wkg[:, ks], D), arrange_w(Wvg[:, ks], D)],
                        axis=1,
                    )
                ),
                "wo": wo_arr,
                "cs_t": cs_full,
                "consts": np.ascontiguousarray(
                    np.concatenate(
                        [
                            wqsum[qs].reshape(G, 128).T,
                            wksum[ks][:, None],
                            wvsum[ks][:, None],
                            bq_full[qs].reshape(G, 128).T,
                            bk_full[ks][:, None],
                            bv_full[ks][:, None],
                        ],
                        axis=1,
                    ).astype(np.float32)
                ),
            }
        )
    return in_maps, has_bias


def kernel(x, ln_gamma, ln_beta, Wq, Wk, Wv, Wo):
    from concourse.bass_utils import run_bass_kernel_spmd

    in_maps, has_bias = make_in_maps(x, ln_gamma, ln_beta, Wq, Wk, Wv, Wo)
    nc = get_program(has_bias)
    res = run_bass_kernel_spmd(nc, in_maps, core_ids=list(range(8)))
    outs = [m["outp"] for m in res.results]
    out = np.empty((B, S, H), np.float32)
    for b in range(B):
        out[b] = (outs[NKV * b] + outs[NKV * b + 1]) + (
            outs[NKV * b + 2] + outs[NKV * b + 3]
        )
    return out
